# revision 26
# baseline (speedup 1.0000x reference)
"""Distributed multi-head attention kernel for 8 TRN2 NeuronCores.

Problem: x(4,2048,1024) -> qkv proj (w_qkv 3072x1024) -> 16-head attention
(head_dim 64, softmax) -> out proj (w_out 1024x1024 + b_out).

Sharding: head-parallel. Core c owns heads {2c, 2c+1}: it computes Q/K/V for
those heads over all 8192 tokens, runs attention, then a per-batch AllToAll
(1MB bf16) converts the head-sharded attention output into a token-sharded
layout (256 tokens/core/batch, all 16 heads) for the output projection --
no all-reduce needed. Work is issued per batch so phases pipeline: QKV(b+1)
and outproj(b-1) run under attention(b)'s softmax, which saturates ScalarE.

Per-core dataflow for batch b (all matmuls bf16 with f32 accumulate):
  1. QKV: Q^T/K^T = Wc @ X^T on PE ([128 = 2 heads x 64 dims, tokens] in
     SBUF, scale 1/8 folded into Q); V computed token-major (X chunks as the
     stationary operand) with a ones-column appended per 128-token chunk
     (65-wide blocks). The 4 V token-subtiles share one PSUM bank and
     start=True clears has_written flags bank-wide, so their accumulation
     groups are chained with explicit ordering deps. PSUM->SBUF epilogues
     run on VectorE to keep ScalarE free. Attention for q-tile 0 is
     interleaved with the QKV tiles as K/V chunks become available.
  2. Attention per (head, 512-wide q-tile): S^T tiles [128 k, 512 q] on PE
     (the two heads' matmuls auto-pack into PE row groups 0-63/64-127 and
     run concurrently); exp on ScalarE (PSUM->SBUF bf16, 1536-wide batches
     across 3 PSUM banks -- the end-to-end bottleneck engine); P.V on PE
     with lhsT = V-chunk [128, 65]: the 65th output row accumulates the
     softmax denominators for free. A VectorE copy releases the PV PSUM
     bank; normalization runs off the critical path: the 512 denominators
     are DMA-reshaped across 128 partitions so reciprocal runs ~4
     elements/lane, then partition_broadcast (GpSimd) + multiply (VectorE).
  3. AllToAll over this batch's 8 x 256-token chunks.
  4. Out proj (pipelined one batch behind, so its PE matmuls never
     head-block the in-order engine queues on a pending collective):
     out = O^T.T @ w_out^T + b_out per 128-token tile, bias added on
     VectorE, DMA to the core's output slice.

Measured on 8 axon-tunneled trn2 cores: ~540 us HW exec, rel err 5.2e-3.
"""

import numpy as np
import ml_dtypes

import concourse.bass as bass
import concourse.mybir as mybir
import concourse.tile as tile
from concourse import bacc, bass_utils
from concourse.tile import add_dep_helper

FP32 = mybir.dt.float32
BF16 = mybir.dt.bfloat16
AF = mybir.ActivationFunctionType

N_CORES = 8
B, NTOK, D = 4, 2048, 1024
T = B * NTOK  # 8192 tokens total
NH, HD = 16, 64
HL = NH // N_CORES  # 2 heads per core
SCALE = float(HD) ** -0.5  # 0.125
TN = 512  # token tile for QKV / q tile for attention
NT = T // TN  # 16
KC = D // 128  # 8 contraction chunks for projections
KT = NTOK // 128  # 16 k-chunks per batch in attention
TPB = NTOK // N_CORES  # 256 tokens per (core, batch) after A2A
TPC = T // N_CORES  # 1024 tokens per core total
WCOLS = 3 * HL * HD  # 384 qkv output dims per core


def build_nc(debug=False):
    nc = bacc.Bacc(
        "TRN2", target_bir_lowering=False, debug=False, num_devices=N_CORES
    )
    xt = nc.dram_tensor("xt", [D, T], BF16, kind="ExternalInput").ap()
    wt = nc.dram_tensor("wt", [D, WCOLS], BF16, kind="ExternalInput").ap()
    wo = nc.dram_tensor("wo", [D, D], BF16, kind="ExternalInput").ap()
    bias = nc.dram_tensor("bias", [1, D], FP32, kind="ExternalInput").ap()
    # row r of out = batch r//TPB, token (core * TPB + r % TPB) of that batch
    out = nc.dram_tensor("out", [TPC, D], FP32, kind="ExternalOutput").ap()
    if debug:
        qdump = nc.dram_tensor("qdump", [128, T], FP32, kind="ExternalOutput").ap()
        kdump = nc.dram_tensor("kdump", [128, T], FP32, kind="ExternalOutput").ap()
        vdump = nc.dram_tensor(
            "vdump", [128, (T // 128) * HL * 65], FP32, kind="ExternalOutput"
        ).ap()
        adump = nc.dram_tensor(
            "adump", [N_CORES, HL * HD, TPB], FP32, kind="ExternalOutput"
        ).ap()

    with tile.TileContext(nc) as tc:
        with (
            tc.tile_pool(name="const", bufs=1) as const,
            tc.tile_pool(name="xin", bufs=24) as xin,
            tc.tile_pool(name="vtp", bufs=3) as vtp,
            tc.tile_pool(name="probs", bufs=6) as probs,
            tc.tile_pool(name="norm", bufs=6) as norm,
            tc.tile_pool(name="ot", bufs=6) as otp,
            tc.tile_pool(name="osb", bufs=2) as osbp,
            tc.tile_pool(name="fin", bufs=4) as fin,
            tc.tile_pool(name="psum", bufs=2, space="PSUM") as psum,
            tc.tile_pool(name="dram", bufs=1, space="DRAM") as dram,
        ):
            # ---- persistent SBUF state ----
            w_sb = const.tile([128, KC * WCOLS], BF16)
            nc.sync.dma_start(
                w_sb[:].rearrange("p (kc j) -> p kc j", kc=KC),
                wt.rearrange("(kc p) j -> p kc j", p=128),
            )
            wo_sb = const.tile([128, KC * D], BF16)
            nc.sync.dma_start(
                wo_sb[:].rearrange("p (kc n) -> p kc n", kc=KC),
                wo.rearrange("(kc p) n -> p kc n", p=128),
            )
            b_row = const.tile([1, D], FP32)
            nc.sync.dma_start(b_row[:], bias[:])
            bias_sb = const.tile([128, D], FP32)
            nc.gpsimd.partition_broadcast(bias_sb[:], b_row[:])

            q_sb = const.tile([128, T], BF16)  # [2 heads x 64, tokens] scaled
            k_sb = const.tile([128, T], BF16)
            # V token-major: [128 tok-in-chunk, (global chunk, head) x 65]
            v_sb = const.tile([128, (T // 128) * HL * 65], BF16)
            v3 = v_sb[:].rearrange("p (blk e) -> p blk e", e=65)
            nc.vector.memset(v3[:, :, 64:65], 1.0)

            a2a_in = {}
            a2a_out = {}
            for b in range(B):
                a2a_in[b] = dram.tile(
                    [N_CORES, HL * HD, TPB], BF16, name=f"a2a_in{b}"
                )
                a2a_out[b] = dram.tile(
                    [N_CORES, HL * HD, TPB], BF16, name=f"a2a_out{b}"
                )

            def emit_a2a(b):
                nc.gpsimd.collective_compute(
                    "AllToAll",
                    mybir.AluOpType.bypass,
                    replica_groups=[list(range(N_CORES))],
                    ins=[a2a_in[b].opt()],
                    outs=[a2a_out[b].opt()],
                )

            def emit_outproj(b):
                o_sb = osbp.tile([128, N_CORES * TPB], BF16, tag="osb", name="o_sb")
                for i in range(N_CORES):
                    nc.sync.dma_start(
                        o_sb[:, i * TPB : (i + 1) * TPB], a2a_out[b][i, :, :]
                    )
                for m in range(TPB // 128):
                    o_ps = [
                        psum.tile([128, 512], FP32, tag="pv", name=f"o_ps{nh}")
                        for nh in range(2)
                    ]
                    for i in range(N_CORES):
                        for nh in range(2):
                            nc.tensor.matmul(
                                o_ps[nh][:, :],
                                lhsT=o_sb[
                                    :, i * TPB + m * 128 : i * TPB + (m + 1) * 128
                                ],
                                rhs=wo_sb[:, i * D + nh * 512 : i * D + nh * 512 + 512],
                                start=(i == 0),
                                stop=(i == N_CORES - 1),
                            )
                    out_t = fin.tile([128, D], FP32, tag="outt", name="out_t")
                    for nh in range(2):
                        nc.vector.tensor_add(
                            out_t[:, nh * 512 : (nh + 1) * 512],
                            o_ps[nh][:, :],
                            bias_sb[:, nh * 512 : (nh + 1) * 512],
                        )
                    nc.sync.dma_start(
                        out[b * TPB + m * 128 : b * TPB + (m + 1) * 128, :],
                        out_t[:],
                    )

            for b in range(B):
                slot_list = [(kc, h) for kc in range(KT) for h in range(HL)]
                groups = [slot_list[g0 : g0 + 3] for g0 in range(0, len(slot_list), 3)]

                def emit_group(group, pv, qt, b=b):
                    q_off = b * NTOK + qt * TN
                    width = len(group) * 512
                    s_t = psum.tile([128, 1536], FP32, tag="big3", name="s_t")
                    for i, (kc, h) in enumerate(group):
                        nc.tensor.matmul(
                            s_t[:, i * 512 : (i + 1) * 512],
                            lhsT=k_sb[
                                h * 64 : (h + 1) * 64,
                                b * NTOK + kc * 128 : b * NTOK + (kc + 1) * 128,
                            ],
                            rhs=q_sb[h * 64 : (h + 1) * 64, q_off : q_off + TN],
                            start=True,
                            stop=True,
                        )
                    p_t = probs.tile([128, 1536], BF16, tag="p", name="p_t")
                    nc.scalar.activation(p_t[:, 0:width], s_t[:, 0:width], AF.Exp)
                    for i, (kc, h) in enumerate(group):
                        gc = b * KT + kc
                        nc.tensor.matmul(
                            pv[h][0:65, :],
                            lhsT=v3[:, gc * HL + h, :],
                            rhs=p_t[:, i * 512 : (i + 1) * 512],
                            start=(kc == 0),
                            stop=(kc == KT - 1),
                        )

                def finish_qt(pv, qt, b=b):
                    for h in range(HL):
                        # single copy releases the PV PSUM bank; the rest of
                        # the normalize chain runs on SBUF off the fast path
                        o_c = norm.tile([65, 512], FP32, tag="oc", name="o_c")
                        nc.vector.tensor_copy(o_c[:], pv[h][0:65, :])
                        # reciprocal on one partition is ~3.3us (512 sequential
                        # elements); DMA-reshape the 512 denominators across
                        # 128 partitions so it runs in ~4 elements/lane
                        rs = norm.tile([128, 4], FP32, tag="rs", name="rs")
                        nc.sync.dma_start(rs[:], o_c[64:65, :])
                        rr = norm.tile([128, 4], FP32, tag="rr", name="rr")
                        nc.vector.reciprocal(rr[:], rs[:])
                        rec = norm.tile([1, 512], FP32, tag="rec", name="rec")
                        nc.sync.dma_start(rec[:], rr[:])
                        bc = norm.tile([64, 512], FP32, tag="bc", name="bc")
                        nc.gpsimd.partition_broadcast(bc[:], rec[:])
                        o_t = otp.tile([64, 512], BF16, tag="o", name="o_t")
                        nc.vector.tensor_mul(o_t[:], o_c[0:64, :], bc[:])
                        nc.sync.dma_start(
                            a2a_in[b][
                                2 * qt : 2 * qt + 2, h * 64 : (h + 1) * 64, :
                            ].rearrange("j p e -> p j e"),
                            o_t[:].rearrange("p (j e) -> p j e", j=2),
                        )

                # ---- QKV for this batch, interleaved with qt=0 attention ----
                pv0 = None
                g_next = 0
                for i, t in enumerate(range(4 * b, 4 * b + 4)):
                    y_ps = psum.tile([128, 1536], FP32, tag="big3", name="y_ps")
                    xts = []
                    for kc in range(KC):
                        x_t = xin.tile([128, TN], BF16, tag="xt", name="x_t")
                        nc.sync.dma_start(
                            x_t[:],
                            xt[kc * 128 : (kc + 1) * 128, t * TN : (t + 1) * TN],
                        )
                        xts.append(x_t)
                    for kc in range(KC):
                        st, sp = kc == 0, kc == KC - 1
                        for m in range(2):  # Q then K, transposed layout
                            nc.tensor.matmul(
                                y_ps[:, m * 512 : (m + 1) * 512],
                                lhsT=w_sb[
                                    :,
                                    kc * WCOLS + m * 128 : kc * WCOLS + (m + 1) * 128,
                                ],
                                rhs=xts[kc][:],
                                start=st,
                                stop=sp,
                            )
                    # V natural layout: 4 token subtiles share one PSUM bank;
                    # start=True clears has_written flags bank-wide, so chain
                    # ordering deps so each accumulation group finishes before
                    # the next begins.
                    prev = None
                    for s in range(4):
                        for kc in range(KC):
                            st, sp = kc == 0, kc == KC - 1
                            mm = nc.tensor.matmul(
                                y_ps[:, 1024 + s * 128 : 1024 + (s + 1) * 128],
                                lhsT=xts[kc][:, s * 128 : (s + 1) * 128],
                                rhs=w_sb[:, kc * WCOLS + 256 : kc * WCOLS + WCOLS],
                                start=st,
                                stop=sp,
                            )
                            if prev is not None:
                                add_dep_helper(
                                    mm.ins, prev.ins, sync=False,
                                    reason="bank flag-clear order",
                                )
                            prev = mm
                    # epilogues on VectorE (keep ScalarE free for exp)
                    nc.vector.tensor_scalar_mul(
                        q_sb[:, t * TN : (t + 1) * TN], y_ps[:, 0:512], SCALE
                    )
                    nc.vector.tensor_copy(
                        k_sb[:, t * TN : (t + 1) * TN], y_ps[:, 512:1024]
                    )
                    nc.vector.tensor_copy(
                        v3[:, (t * 4) * HL : (t * 4 + 4) * HL, 0:64],
                        y_ps[:, 1024:1536]
                        .rearrange("p (s hd) -> p s hd", s=4)
                        .rearrange("p s (h d) -> p (s h) d", h=HL),
                    )
                    # early attention for qt=0 as K/V chunks become ready
                    if pv0 is None:
                        pv0 = [
                            psum.tile([128, 512], FP32, tag="pv", name=f"pv{h}")
                            for h in range(HL)
                        ]
                    avail = 4 * (i + 1)
                    while g_next < len(groups) and all(
                        kc < avail for kc, _ in groups[g_next]
                    ):
                        emit_group(groups[g_next], pv0, 0)
                        g_next += 1
                finish_qt(pv0, 0)

                # ---- attention for remaining q tiles ----
                for qt in range(1, NTOK // TN):
                    pv = [
                        psum.tile([128, 512], FP32, tag="pv", name=f"pv{h}")
                        for h in range(HL)
                    ]
                    for g in groups:
                        emit_group(g, pv, qt)
                    finish_qt(pv, qt)

                # ---- output projection for batch b-1 (its A2A is long
                # done) -- emitted here so its PE matmuls never head-block
                # the in-order engine streams on a pending collective ----
                if b >= 1:
                    emit_outproj(b - 1)

                emit_a2a(b)

            emit_outproj(B - 1)

            if debug:
                for t in range(NT):
                    d1 = fin.tile([128, TN], FP32, tag="outt", name="d1")
                    nc.vector.tensor_copy(d1[:], q_sb[:, t * TN : (t + 1) * TN])
                    nc.sync.dma_start(qdump[:, t * TN : (t + 1) * TN], d1[:])
                    d2 = fin.tile([128, TN], FP32, tag="outt", name="d2")
                    nc.vector.tensor_copy(d2[:], k_sb[:, t * TN : (t + 1) * TN])
                    nc.sync.dma_start(kdump[:, t * TN : (t + 1) * TN], d2[:])
                nv = (T // 128) * HL * 65
                for j in range(0, nv, 1024):
                    wdt = min(1024, nv - j)
                    d3 = fin.tile([128, 1024], FP32, tag="outt", name="d3")
                    nc.vector.tensor_copy(d3[:, 0:wdt], v_sb[:, j : j + wdt])
                    nc.sync.dma_start(vdump[:, j : j + wdt], d3[:, 0:wdt])
                for i in range(N_CORES):
                    d4 = fin.tile([128, TPB], BF16, tag="outt", name="d4")
                    nc.sync.dma_start(d4[:], a2a_in[0][i, :, :])
                    d5 = fin.tile([128, TPB], FP32, tag="outt", name="d5")
                    nc.vector.tensor_copy(d5[:], d4[:])
                    nc.sync.dma_start(adump[i, :, :], d5[:])

    nc.compile()
    return nc


_NC_CACHE = None


def _get_nc():
    global _NC_CACHE
    if _NC_CACHE is None:
        _NC_CACHE = build_nc()
    return _NC_CACHE


def make_in_maps(x, w_qkv, w_out, b_out):
    x = np.asarray(x, dtype=np.float32)
    w_qkv = np.asarray(w_qkv, dtype=np.float32)
    w_out = np.asarray(w_out, dtype=np.float32)
    b_out = np.asarray(b_out, dtype=np.float32)

    xt_np = np.ascontiguousarray(x.reshape(T, D).T).astype(ml_dtypes.bfloat16)
    wo_np = np.ascontiguousarray(w_out.T).astype(ml_dtypes.bfloat16)
    b_np = np.ascontiguousarray(b_out.reshape(1, D))

    in_maps = []
    for c in range(N_CORES):
        rows = []
        for sec in range(3):  # q, k, v sections of w_qkv
            for hh in range(HL):
                h = HL * c + hh
                rows.append(w_qkv[sec * D + h * HD : sec * D + (h + 1) * HD, :])
        wt_np = np.ascontiguousarray(np.concatenate(rows, 0).T).astype(
            ml_dtypes.bfloat16
        )  # (1024, 384)
        in_maps.append({"xt": xt_np, "wt": wt_np, "wo": wo_np, "bias": b_np})
    return in_maps


def kernel(x, w_qkv, w_out, b_out, _trace=False, _tmpdir=None):
    in_maps = make_in_maps(x, w_qkv, w_out, b_out)
    nc = _get_nc()
    res = bass_utils.run_bass_kernel_spmd(
        nc, in_maps, core_ids=list(range(N_CORES)), trace=_trace, tmpdir=_tmpdir
    )
    # core j out row r = b*TPB + u -> global token b*NTOK + j*TPB + u
    stacked = np.stack([res.results[c]["out"] for c in range(N_CORES)], 0)
    full = stacked.reshape(N_CORES, B, TPB, D).transpose(1, 0, 2, 3)
    kernel.last_result = res
    return np.ascontiguousarray(full.reshape(B, NTOK, D)).astype(np.float32)


# revision 28
# speedup vs baseline: 1.0574x; 1.0574x over previous
"""Distributed multi-head attention kernel for 8 TRN2 NeuronCores.

Problem: x(4,2048,1024) -> qkv proj (w_qkv 3072x1024) -> 16-head attention
(head_dim 64, softmax) -> out proj (w_out 1024x1024 + b_out).

Sharding: head-parallel. Core c owns heads {2c, 2c+1}: it computes Q/K/V for
those heads over all 8192 tokens, runs attention, then a per-batch AllToAll
(1MB bf16) converts the head-sharded attention output into a token-sharded
layout (256 tokens/core/batch, all 16 heads) for the output projection --
no all-reduce needed. Work is issued per batch so phases pipeline: QKV(b+1)
and outproj(b-1) run under attention(b)'s softmax, which saturates ScalarE.

Per-core dataflow for batch b (all matmuls bf16 with f32 accumulate):
  1. QKV: Q^T/K^T = Wc @ X^T on PE ([128 = 2 heads x 64 dims, tokens] in
     SBUF, scale 1/8 folded into Q); V computed token-major (X chunks as the
     stationary operand) with a ones-column appended per 128-token chunk
     (65-wide blocks). The 4 V token-subtiles share one PSUM bank and
     start=True clears has_written flags bank-wide, so their accumulation
     groups are chained with explicit ordering deps. PSUM->SBUF epilogues
     run on VectorE to keep ScalarE free. Attention for q-tile 0 is
     interleaved with the QKV tiles as K/V chunks become available.
  2. Attention per (head, 512-wide q-tile): S^T tiles [128 k, 512 q] on PE
     (the two heads' matmuls auto-pack into PE row groups 0-63/64-127 and
     run concurrently); exp on ScalarE (PSUM->SBUF bf16, 1536-wide batches
     across 3 PSUM banks -- the end-to-end bottleneck engine); P.V on PE
     with lhsT = V-chunk [128, 65]: the 65th output row accumulates the
     softmax denominators for free. A VectorE copy releases the PV PSUM
     bank; normalization runs off the critical path: the 512 denominators
     are DMA-reshaped across 128 partitions so reciprocal runs ~4
     elements/lane, then partition_broadcast (GpSimd) + multiply (VectorE).
  3. AllToAll over this batch's 8 x 256-token chunks.
  4. Out proj (pipelined one batch behind, so its PE matmuls never
     head-block the in-order engine queues on a pending collective):
     out = O^T.T @ w_out^T + b_out per 128-token tile, bias added on
     VectorE, DMA to the core's output slice.

Measured on 8 axon-tunneled trn2 cores: ~540 us HW exec, rel err 5.2e-3.
"""

import numpy as np
import ml_dtypes

import concourse.bass as bass
import concourse.mybir as mybir
import concourse.tile as tile
from concourse import bacc, bass_utils
from concourse.tile import add_dep_helper

FP32 = mybir.dt.float32
BF16 = mybir.dt.bfloat16
AF = mybir.ActivationFunctionType

N_CORES = 8
B, NTOK, D = 4, 2048, 1024
T = B * NTOK  # 8192 tokens total
NH, HD = 16, 64
HL = NH // N_CORES  # 2 heads per core
SCALE = float(HD) ** -0.5  # 0.125
TN = 512  # token tile for QKV / q tile for attention
NT = T // TN  # 16
KC = D // 128  # 8 contraction chunks for projections
KT = NTOK // 128  # 16 k-chunks per batch in attention
TPB = NTOK // N_CORES  # 256 tokens per (core, batch) after A2A
TPC = T // N_CORES  # 1024 tokens per core total
WCOLS = 3 * HL * HD  # 384 qkv output dims per core


def build_nc(debug=False):
    nc = bacc.Bacc(
        "TRN2", target_bir_lowering=False, debug=False, num_devices=N_CORES
    )
    xt = nc.dram_tensor("xt", [D, T], BF16, kind="ExternalInput").ap()
    wt = nc.dram_tensor("wt", [D, WCOLS], BF16, kind="ExternalInput").ap()
    wo = nc.dram_tensor("wo", [D, D], BF16, kind="ExternalInput").ap()
    bias = nc.dram_tensor("bias", [1, D], FP32, kind="ExternalInput").ap()
    # row r of out = batch r//TPB, token (core * TPB + r % TPB) of that batch
    out = nc.dram_tensor("out", [TPC, D], FP32, kind="ExternalOutput").ap()
    if debug:
        qdump = nc.dram_tensor("qdump", [128, T], FP32, kind="ExternalOutput").ap()
        kdump = nc.dram_tensor("kdump", [128, T], FP32, kind="ExternalOutput").ap()
        vdump = nc.dram_tensor(
            "vdump", [128, (T // 128) * HL * 65], FP32, kind="ExternalOutput"
        ).ap()
        adump = nc.dram_tensor(
            "adump", [N_CORES, HL * HD, TPB], FP32, kind="ExternalOutput"
        ).ap()

    with tile.TileContext(nc) as tc:
        with (
            tc.tile_pool(name="const", bufs=1) as const,
            tc.tile_pool(name="xin", bufs=24) as xin,
            tc.tile_pool(name="vtp", bufs=3) as vtp,
            tc.tile_pool(name="probs", bufs=6) as probs,
            tc.tile_pool(name="norm", bufs=6) as norm,
            tc.tile_pool(name="ot", bufs=6) as otp,
            tc.tile_pool(name="osb", bufs=2) as osbp,
            tc.tile_pool(name="fin", bufs=4) as fin,
            tc.tile_pool(name="psum", bufs=2, space="PSUM") as psum,
            tc.tile_pool(name="dram", bufs=1, space="DRAM") as dram,
        ):
            # ---- persistent SBUF state ----
            w_sb = const.tile([128, KC * WCOLS], BF16)
            nc.sync.dma_start(
                w_sb[:].rearrange("p (kc j) -> p kc j", kc=KC),
                wt.rearrange("(kc p) j -> p kc j", p=128),
            )
            wo_sb = const.tile([128, KC * D], BF16)
            nc.sync.dma_start(
                wo_sb[:].rearrange("p (kc n) -> p kc n", kc=KC),
                wo.rearrange("(kc p) n -> p kc n", p=128),
            )
            b_row = const.tile([1, D], FP32)
            nc.sync.dma_start(b_row[:], bias[:])
            bias_sb = const.tile([128, D], FP32)
            nc.gpsimd.partition_broadcast(bias_sb[:], b_row[:])

            q_sb = const.tile([128, T], BF16)  # [2 heads x 64, tokens] scaled
            k_sb = const.tile([128, T], BF16)
            # V token-major: [128 tok-in-chunk, (global chunk, head) x 65]
            v_sb = const.tile([128, (T // 128) * HL * 65], BF16)
            v3 = v_sb[:].rearrange("p (blk e) -> p blk e", e=65)
            nc.vector.memset(v3[:, :, 64:65], 1.0)

            a2a_in = {}
            a2a_out = {}
            for b in range(B):
                a2a_in[b] = dram.tile(
                    [N_CORES, HL * HD, TPB], BF16, name=f"a2a_in{b}"
                )
                a2a_out[b] = dram.tile(
                    [N_CORES, HL * HD, TPB], BF16, name=f"a2a_out{b}"
                )

            def emit_a2a(b):
                nc.gpsimd.collective_compute(
                    "AllToAll",
                    mybir.AluOpType.bypass,
                    replica_groups=[list(range(N_CORES))],
                    ins=[a2a_in[b].opt()],
                    outs=[a2a_out[b].opt()],
                )

            def emit_outproj(b):
                o_sb = osbp.tile([128, N_CORES * TPB], BF16, tag="osb", name="o_sb")
                for i in range(N_CORES):
                    nc.sync.dma_start(
                        o_sb[:, i * TPB : (i + 1) * TPB], a2a_out[b][i, :, :]
                    )
                for m in range(TPB // 128):
                    o_ps = [
                        psum.tile([128, 512], FP32, tag="pv", name=f"o_ps{nh}")
                        for nh in range(2)
                    ]
                    for i in range(N_CORES):
                        for nh in range(2):
                            nc.tensor.matmul(
                                o_ps[nh][:, :],
                                lhsT=o_sb[
                                    :, i * TPB + m * 128 : i * TPB + (m + 1) * 128
                                ],
                                rhs=wo_sb[:, i * D + nh * 512 : i * D + nh * 512 + 512],
                                start=(i == 0),
                                stop=(i == N_CORES - 1),
                            )
                    out_t = fin.tile([128, D], FP32, tag="outt", name="out_t")
                    for nh in range(2):
                        nc.vector.tensor_add(
                            out_t[:, nh * 512 : (nh + 1) * 512],
                            o_ps[nh][:, :],
                            bias_sb[:, nh * 512 : (nh + 1) * 512],
                        )
                    nc.sync.dma_start(
                        out[b * TPB + m * 128 : b * TPB + (m + 1) * 128, :],
                        out_t[:],
                    )

            for b in range(B):
                slot_list = [(kc, h) for kc in range(KT) for h in range(HL)]
                groups = [slot_list[g0 : g0 + 3] for g0 in range(0, len(slot_list), 3)]

                pending = []  # (group, p_t, pv) with S+exp emitted, PV not

                def emit_pv_flush(b=b):
                    group, p_t, pv = pending.pop(0)
                    for i, (kc, h) in enumerate(group):
                        gc = b * KT + kc
                        nc.tensor.matmul(
                            pv[h][0:65, :],
                            lhsT=v3[:, gc * HL + h, :],
                            rhs=p_t[:, i * 512 : (i + 1) * 512],
                            start=(kc == 0),
                            stop=(kc == KT - 1),
                        )

                def emit_group(group, pv, qt, b=b):
                    # S matmuls + exp for this group; the PV matmuls are
                    # emitted one group later (via pending) so the in-order
                    # PE queue never head-stalls on the exp of its own group
                    q_off = b * NTOK + qt * TN
                    width = len(group) * 512
                    s_t = psum.tile([128, 1536], FP32, tag="big3", name="s_t")
                    for i, (kc, h) in enumerate(group):
                        nc.tensor.matmul(
                            s_t[:, i * 512 : (i + 1) * 512],
                            lhsT=k_sb[
                                h * 64 : (h + 1) * 64,
                                b * NTOK + kc * 128 : b * NTOK + (kc + 1) * 128,
                            ],
                            rhs=q_sb[h * 64 : (h + 1) * 64, q_off : q_off + TN],
                            start=True,
                            stop=True,
                        )
                    p_t = probs.tile([128, 1536], BF16, tag="p", name="p_t")
                    nc.scalar.activation(p_t[:, 0:width], s_t[:, 0:width], AF.Exp)
                    pending.append((group, p_t, pv))
                    while len(pending) > 1:
                        emit_pv_flush()

                def finish_qt(pv, qt, b=b):
                    for h in range(HL):
                        # single copy releases the PV PSUM bank; the rest of
                        # the normalize chain runs on SBUF off the fast path
                        o_c = norm.tile([65, 512], FP32, tag="oc", name="o_c")
                        nc.vector.tensor_copy(o_c[:], pv[h][0:65, :])
                        # reciprocal on one partition is ~3.3us (512 sequential
                        # elements); DMA-reshape the 512 denominators across
                        # 128 partitions so it runs in ~4 elements/lane
                        rs = norm.tile([128, 4], FP32, tag="rs", name="rs")
                        nc.sync.dma_start(rs[:], o_c[64:65, :])
                        rr = norm.tile([128, 4], FP32, tag="rr", name="rr")
                        nc.vector.reciprocal(rr[:], rs[:])
                        rec = norm.tile([1, 512], FP32, tag="rec", name="rec")
                        nc.sync.dma_start(rec[:], rr[:])
                        bc = norm.tile([64, 512], FP32, tag="bc", name="bc")
                        nc.gpsimd.partition_broadcast(bc[:], rec[:])
                        o_t = otp.tile([64, 512], BF16, tag="o", name="o_t")
                        nc.vector.tensor_mul(o_t[:], o_c[0:64, :], bc[:])
                        nc.sync.dma_start(
                            a2a_in[b][
                                2 * qt : 2 * qt + 2, h * 64 : (h + 1) * 64, :
                            ].rearrange("j p e -> p j e"),
                            o_t[:].rearrange("p (j e) -> p j e", j=2),
                        )

                # ---- QKV for this batch, interleaved with qt=0 attention ----
                pv0 = None
                g_next = 0
                for i, t in enumerate(range(4 * b, 4 * b + 4)):
                    y_ps = psum.tile([128, 1536], FP32, tag="big3", name="y_ps")
                    xts = []
                    for kc in range(KC):
                        x_t = xin.tile([128, TN], BF16, tag="xt", name="x_t")
                        nc.sync.dma_start(
                            x_t[:],
                            xt[kc * 128 : (kc + 1) * 128, t * TN : (t + 1) * TN],
                        )
                        xts.append(x_t)
                    for kc in range(KC):
                        st, sp = kc == 0, kc == KC - 1
                        for m in range(2):  # Q then K, transposed layout
                            nc.tensor.matmul(
                                y_ps[:, m * 512 : (m + 1) * 512],
                                lhsT=w_sb[
                                    :,
                                    kc * WCOLS + m * 128 : kc * WCOLS + (m + 1) * 128,
                                ],
                                rhs=xts[kc][:],
                                start=st,
                                stop=sp,
                            )
                    # V natural layout: 4 token subtiles share one PSUM bank;
                    # start=True clears has_written flags bank-wide, so chain
                    # ordering deps so each accumulation group finishes before
                    # the next begins.
                    prev = None
                    for s in range(4):
                        for kc in range(KC):
                            st, sp = kc == 0, kc == KC - 1
                            mm = nc.tensor.matmul(
                                y_ps[:, 1024 + s * 128 : 1024 + (s + 1) * 128],
                                lhsT=xts[kc][:, s * 128 : (s + 1) * 128],
                                rhs=w_sb[:, kc * WCOLS + 256 : kc * WCOLS + WCOLS],
                                start=st,
                                stop=sp,
                            )
                            if prev is not None:
                                add_dep_helper(
                                    mm.ins, prev.ins, sync=False,
                                    reason="bank flag-clear order",
                                )
                            prev = mm
                    # epilogues on VectorE (keep ScalarE free for exp)
                    nc.vector.tensor_scalar_mul(
                        q_sb[:, t * TN : (t + 1) * TN], y_ps[:, 0:512], SCALE
                    )
                    nc.vector.tensor_copy(
                        k_sb[:, t * TN : (t + 1) * TN], y_ps[:, 512:1024]
                    )
                    nc.vector.tensor_copy(
                        v3[:, (t * 4) * HL : (t * 4 + 4) * HL, 0:64],
                        y_ps[:, 1024:1536]
                        .rearrange("p (s hd) -> p s hd", s=4)
                        .rearrange("p s (h d) -> p (s h) d", h=HL),
                    )
                    # early attention for qt=0 as K/V chunks become ready
                    if pv0 is None:
                        pv0 = [
                            psum.tile([128, 512], FP32, tag="pv", name=f"pv{h}")
                            for h in range(HL)
                        ]
                    avail = 4 * (i + 1)
                    while g_next < len(groups) and all(
                        kc < avail for kc, _ in groups[g_next]
                    ):
                        emit_group(groups[g_next], pv0, 0)
                        g_next += 1
                fin_q = [(pv0, 0)]

                # ---- attention for remaining q tiles ----
                for qt in range(1, NTOK // TN):
                    pv = [
                        psum.tile([128, 512], FP32, tag="pv", name=f"pv{h}")
                        for h in range(HL)
                    ]
                    for gi, g in enumerate(groups):
                        emit_group(g, pv, qt)
                        if gi == 0 and fin_q:
                            # the previous q-tile's last PV group was just
                            # flushed by the pipeline; it can be finished now
                            finish_qt(*fin_q.pop(0))
                    fin_q.append((pv, qt))
                while pending:
                    emit_pv_flush()
                for f in fin_q:
                    finish_qt(*f)
                fin_q = []

                # ---- output projection for batch b-1 (its A2A is long
                # done) -- emitted here so its PE matmuls never head-block
                # the in-order engine streams on a pending collective ----
                if b >= 1:
                    emit_outproj(b - 1)

                emit_a2a(b)

            emit_outproj(B - 1)

            if debug:
                for t in range(NT):
                    d1 = fin.tile([128, TN], FP32, tag="outt", name="d1")
                    nc.vector.tensor_copy(d1[:], q_sb[:, t * TN : (t + 1) * TN])
                    nc.sync.dma_start(qdump[:, t * TN : (t + 1) * TN], d1[:])
                    d2 = fin.tile([128, TN], FP32, tag="outt", name="d2")
                    nc.vector.tensor_copy(d2[:], k_sb[:, t * TN : (t + 1) * TN])
                    nc.sync.dma_start(kdump[:, t * TN : (t + 1) * TN], d2[:])
                nv = (T // 128) * HL * 65
                for j in range(0, nv, 1024):
                    wdt = min(1024, nv - j)
                    d3 = fin.tile([128, 1024], FP32, tag="outt", name="d3")
                    nc.vector.tensor_copy(d3[:, 0:wdt], v_sb[:, j : j + wdt])
                    nc.sync.dma_start(vdump[:, j : j + wdt], d3[:, 0:wdt])
                for i in range(N_CORES):
                    d4 = fin.tile([128, TPB], BF16, tag="outt", name="d4")
                    nc.sync.dma_start(d4[:], a2a_in[0][i, :, :])
                    d5 = fin.tile([128, TPB], FP32, tag="outt", name="d5")
                    nc.vector.tensor_copy(d5[:], d4[:])
                    nc.sync.dma_start(adump[i, :, :], d5[:])

    nc.compile()
    return nc


_NC_CACHE = None


def _get_nc():
    global _NC_CACHE
    if _NC_CACHE is None:
        _NC_CACHE = build_nc()
    return _NC_CACHE


def make_in_maps(x, w_qkv, w_out, b_out):
    x = np.asarray(x, dtype=np.float32)
    w_qkv = np.asarray(w_qkv, dtype=np.float32)
    w_out = np.asarray(w_out, dtype=np.float32)
    b_out = np.asarray(b_out, dtype=np.float32)

    xt_np = np.ascontiguousarray(x.reshape(T, D).T).astype(ml_dtypes.bfloat16)
    wo_np = np.ascontiguousarray(w_out.T).astype(ml_dtypes.bfloat16)
    b_np = np.ascontiguousarray(b_out.reshape(1, D))

    in_maps = []
    for c in range(N_CORES):
        rows = []
        for sec in range(3):  # q, k, v sections of w_qkv
            for hh in range(HL):
                h = HL * c + hh
                rows.append(w_qkv[sec * D + h * HD : sec * D + (h + 1) * HD, :])
        wt_np = np.ascontiguousarray(np.concatenate(rows, 0).T).astype(
            ml_dtypes.bfloat16
        )  # (1024, 384)
        in_maps.append({"xt": xt_np, "wt": wt_np, "wo": wo_np, "bias": b_np})
    return in_maps


def kernel(x, w_qkv, w_out, b_out, _trace=False, _tmpdir=None):
    in_maps = make_in_maps(x, w_qkv, w_out, b_out)
    nc = _get_nc()
    res = bass_utils.run_bass_kernel_spmd(
        nc, in_maps, core_ids=list(range(N_CORES)), trace=_trace, tmpdir=_tmpdir
    )
    # core j out row r = b*TPB + u -> global token b*NTOK + j*TPB + u
    stacked = np.stack([res.results[c]["out"] for c in range(N_CORES)], 0)
    full = stacked.reshape(N_CORES, B, TPB, D).transpose(1, 0, 2, 3)
    kernel.last_result = res
    return np.ascontiguousarray(full.reshape(B, NTOK, D)).astype(np.float32)


# revision 29
# speedup vs baseline: 1.0618x; 1.0041x over previous
"""Distributed multi-head attention kernel for 8 TRN2 NeuronCores.

Problem: x(4,2048,1024) -> qkv proj (w_qkv 3072x1024) -> 16-head attention
(head_dim 64, softmax) -> out proj (w_out 1024x1024 + b_out).

Sharding: head-parallel. Core c owns heads {2c, 2c+1}: it computes Q/K/V for
those heads over all 8192 tokens, runs attention, then a per-batch AllToAll
(1MB bf16) converts the head-sharded attention output into a token-sharded
layout (256 tokens/core/batch, all 16 heads) for the output projection --
no all-reduce needed. Work is issued per batch so phases pipeline: QKV(b+1)
and outproj(b-1) run under attention(b)'s softmax, which saturates ScalarE.

Per-core dataflow for batch b (all matmuls bf16 with f32 accumulate):
  1. QKV: Q^T/K^T = Wc @ X^T on PE ([128 = 2 heads x 64 dims, tokens] in
     SBUF, scale 1/8 folded into Q); V computed token-major (X chunks as the
     stationary operand) with a ones-column appended per 128-token chunk
     (65-wide blocks). The 4 V token-subtiles share one PSUM bank and
     start=True clears has_written flags bank-wide, so their accumulation
     groups are chained with explicit ordering deps. PSUM->SBUF epilogues
     run on VectorE to keep ScalarE free. Attention for q-tile 0 is
     interleaved with the QKV tiles as K/V chunks become available.
  2. Attention per (head, 512-wide q-tile): S^T tiles [128 k, 512 q] on PE
     (the two heads' matmuls auto-pack into PE row groups 0-63/64-127 and
     run concurrently); exp on ScalarE (PSUM->SBUF bf16, 1536-wide batches
     across 3 PSUM banks); P.V on PE with lhsT = V-chunk [128, 65]: the
     65th output row accumulates the softmax denominators for free. The PV
     matmuls are software-pipelined one exp-group behind the S matmuls
     (including across q-tile boundaries) so the in-order PE queue never
     head-stalls waiting for its own group's exp. A VectorE copy releases the PV PSUM
     bank; normalization runs off the critical path: the 512 denominators
     are DMA-reshaped across 128 partitions so reciprocal runs ~4
     elements/lane, then partition_broadcast (GpSimd) + multiply (VectorE).
  3. AllToAll over this batch's 8 x 256-token chunks.
  4. Out proj (pipelined one batch behind, so its PE matmuls never
     head-block the in-order engine queues on a pending collective):
     out = O^T.T @ w_out^T + b_out per 128-token tile, bias added on
     VectorE, DMA to the core's output slice.

Measured on 8 axon-tunneled trn2 cores: ~525 us HW exec, rel err 5.2e-3.
"""

import numpy as np
import ml_dtypes

import concourse.bass as bass
import concourse.mybir as mybir
import concourse.tile as tile
from concourse import bacc, bass_utils
from concourse.tile import add_dep_helper

FP32 = mybir.dt.float32
BF16 = mybir.dt.bfloat16
AF = mybir.ActivationFunctionType

N_CORES = 8
B, NTOK, D = 4, 2048, 1024
T = B * NTOK  # 8192 tokens total
NH, HD = 16, 64
HL = NH // N_CORES  # 2 heads per core
SCALE = float(HD) ** -0.5  # 0.125
TN = 512  # token tile for QKV / q tile for attention
NT = T // TN  # 16
KC = D // 128  # 8 contraction chunks for projections
KT = NTOK // 128  # 16 k-chunks per batch in attention
TPB = NTOK // N_CORES  # 256 tokens per (core, batch) after A2A
TPC = T // N_CORES  # 1024 tokens per core total
WCOLS = 3 * HL * HD  # 384 qkv output dims per core


def build_nc(debug=False):
    nc = bacc.Bacc(
        "TRN2", target_bir_lowering=False, debug=False, num_devices=N_CORES
    )
    xt = nc.dram_tensor("xt", [D, T], BF16, kind="ExternalInput").ap()
    wt = nc.dram_tensor("wt", [D, WCOLS], BF16, kind="ExternalInput").ap()
    wo = nc.dram_tensor("wo", [D, D], BF16, kind="ExternalInput").ap()
    bias = nc.dram_tensor("bias", [1, D], FP32, kind="ExternalInput").ap()
    # row r of out = batch r//TPB, token (core * TPB + r % TPB) of that batch
    out = nc.dram_tensor("out", [TPC, D], FP32, kind="ExternalOutput").ap()
    if debug:
        qdump = nc.dram_tensor("qdump", [128, T], FP32, kind="ExternalOutput").ap()
        kdump = nc.dram_tensor("kdump", [128, T], FP32, kind="ExternalOutput").ap()
        vdump = nc.dram_tensor(
            "vdump", [128, (T // 128) * HL * 65], FP32, kind="ExternalOutput"
        ).ap()
        adump = nc.dram_tensor(
            "adump", [N_CORES, HL * HD, TPB], FP32, kind="ExternalOutput"
        ).ap()

    with tile.TileContext(nc) as tc:
        with (
            tc.tile_pool(name="const", bufs=1) as const,
            tc.tile_pool(name="xin", bufs=24) as xin,
            tc.tile_pool(name="vtp", bufs=3) as vtp,
            tc.tile_pool(name="probs", bufs=6) as probs,
            tc.tile_pool(name="norm", bufs=6) as norm,
            tc.tile_pool(name="ot", bufs=6) as otp,
            tc.tile_pool(name="osb", bufs=2) as osbp,
            tc.tile_pool(name="fin", bufs=4) as fin,
            tc.tile_pool(name="psum", bufs=2, space="PSUM") as psum,
            tc.tile_pool(name="dram", bufs=1, space="DRAM") as dram,
        ):
            # ---- persistent SBUF state ----
            w_sb = const.tile([128, KC * WCOLS], BF16)
            nc.sync.dma_start(
                w_sb[:].rearrange("p (kc j) -> p kc j", kc=KC),
                wt.rearrange("(kc p) j -> p kc j", p=128),
            )
            wo_sb = const.tile([128, KC * D], BF16)
            nc.sync.dma_start(
                wo_sb[:].rearrange("p (kc n) -> p kc n", kc=KC),
                wo.rearrange("(kc p) n -> p kc n", p=128),
            )
            b_row = const.tile([1, D], FP32)
            nc.sync.dma_start(b_row[:], bias[:])
            bias_sb = const.tile([128, D], FP32)
            nc.gpsimd.partition_broadcast(bias_sb[:], b_row[:])

            q_sb = const.tile([128, T], BF16)  # [2 heads x 64, tokens] scaled
            k_sb = const.tile([128, T], BF16)
            # V token-major: [128 tok-in-chunk, (global chunk, head) x 65]
            v_sb = const.tile([128, (T // 128) * HL * 65], BF16)
            v3 = v_sb[:].rearrange("p (blk e) -> p blk e", e=65)
            nc.vector.memset(v3[:, :, 64:65], 1.0)

            a2a_in = {}
            a2a_out = {}
            for b in range(B):
                a2a_in[b] = dram.tile(
                    [N_CORES, HL * HD, TPB], BF16, name=f"a2a_in{b}"
                )
                a2a_out[b] = dram.tile(
                    [N_CORES, HL * HD, TPB], BF16, name=f"a2a_out{b}"
                )

            def emit_a2a(b):
                nc.gpsimd.collective_compute(
                    "AllToAll",
                    mybir.AluOpType.bypass,
                    replica_groups=[list(range(N_CORES))],
                    ins=[a2a_in[b].opt()],
                    outs=[a2a_out[b].opt()],
                )

            def emit_outproj(b):
                o_sb = osbp.tile([128, N_CORES * TPB], BF16, tag="osb", name="o_sb")
                for i in range(N_CORES):
                    nc.sync.dma_start(
                        o_sb[:, i * TPB : (i + 1) * TPB], a2a_out[b][i, :, :]
                    )
                for m in range(TPB // 128):
                    o_ps = [
                        psum.tile([128, 512], FP32, tag="pv", name=f"o_ps{nh}")
                        for nh in range(2)
                    ]
                    for i in range(N_CORES):
                        for nh in range(2):
                            nc.tensor.matmul(
                                o_ps[nh][:, :],
                                lhsT=o_sb[
                                    :, i * TPB + m * 128 : i * TPB + (m + 1) * 128
                                ],
                                rhs=wo_sb[:, i * D + nh * 512 : i * D + nh * 512 + 512],
                                start=(i == 0),
                                stop=(i == N_CORES - 1),
                            )
                    out_t = fin.tile([128, D], FP32, tag="outt", name="out_t")
                    for nh in range(2):
                        nc.vector.tensor_add(
                            out_t[:, nh * 512 : (nh + 1) * 512],
                            o_ps[nh][:, :],
                            bias_sb[:, nh * 512 : (nh + 1) * 512],
                        )
                    nc.sync.dma_start(
                        out[b * TPB + m * 128 : b * TPB + (m + 1) * 128, :],
                        out_t[:],
                    )

            for b in range(B):
                slot_list = [(kc, h) for kc in range(KT) for h in range(HL)]
                groups = [slot_list[g0 : g0 + 3] for g0 in range(0, len(slot_list), 3)]

                pending = []  # (group, p_t, pv) with S+exp emitted, PV not

                def emit_pv_flush(b=b):
                    group, p_t, pv = pending.pop(0)
                    for i, (kc, h) in enumerate(group):
                        gc = b * KT + kc
                        nc.tensor.matmul(
                            pv[h][0:65, :],
                            lhsT=v3[:, gc * HL + h, :],
                            rhs=p_t[:, i * 512 : (i + 1) * 512],
                            start=(kc == 0),
                            stop=(kc == KT - 1),
                        )

                def emit_group(group, pv, qt, b=b):
                    # S matmuls + exp for this group; the PV matmuls are
                    # emitted one group later (via pending) so the in-order
                    # PE queue never head-stalls on the exp of its own group
                    q_off = b * NTOK + qt * TN
                    width = len(group) * 512
                    s_t = psum.tile([128, 1536], FP32, tag="big3", name="s_t")
                    for i, (kc, h) in enumerate(group):
                        nc.tensor.matmul(
                            s_t[:, i * 512 : (i + 1) * 512],
                            lhsT=k_sb[
                                h * 64 : (h + 1) * 64,
                                b * NTOK + kc * 128 : b * NTOK + (kc + 1) * 128,
                            ],
                            rhs=q_sb[h * 64 : (h + 1) * 64, q_off : q_off + TN],
                            start=True,
                            stop=True,
                        )
                    p_t = probs.tile([128, 1536], BF16, tag="p", name="p_t")
                    nc.scalar.activation(p_t[:, 0:width], s_t[:, 0:width], AF.Exp)
                    pending.append((group, p_t, pv))
                    while len(pending) > 1:
                        emit_pv_flush()

                def finish_qt(pv, qt, b=b):
                    for h in range(HL):
                        # single copy releases the PV PSUM bank; the rest of
                        # the normalize chain runs on SBUF off the fast path
                        o_c = norm.tile([65, 512], FP32, tag="oc", name="o_c")
                        nc.vector.tensor_copy(o_c[:], pv[h][0:65, :])
                        # reciprocal on one partition is ~3.3us (512 sequential
                        # elements); DMA-reshape the 512 denominators across
                        # 128 partitions so it runs in ~4 elements/lane
                        rs = norm.tile([128, 4], FP32, tag="rs", name="rs")
                        nc.sync.dma_start(rs[:], o_c[64:65, :])
                        rr = norm.tile([128, 4], FP32, tag="rr", name="rr")
                        nc.vector.reciprocal(rr[:], rs[:])
                        rec = norm.tile([1, 512], FP32, tag="rec", name="rec")
                        nc.sync.dma_start(rec[:], rr[:])
                        bc = norm.tile([64, 512], FP32, tag="bc", name="bc")
                        nc.gpsimd.partition_broadcast(bc[:], rec[:])
                        o_t = otp.tile([64, 512], BF16, tag="o", name="o_t")
                        nc.vector.tensor_mul(o_t[:], o_c[0:64, :], bc[:])
                        nc.sync.dma_start(
                            a2a_in[b][
                                2 * qt : 2 * qt + 2, h * 64 : (h + 1) * 64, :
                            ].rearrange("j p e -> p j e"),
                            o_t[:].rearrange("p (j e) -> p j e", j=2),
                        )

                # ---- QKV for this batch, interleaved with qt=0 attention ----
                pv0 = None
                g_next = 0
                for i, t in enumerate(range(4 * b, 4 * b + 4)):
                    y_ps = psum.tile([128, 1536], FP32, tag="big3", name="y_ps")
                    xts = []
                    for kc in range(KC):
                        x_t = xin.tile([128, TN], BF16, tag="xt", name="x_t")
                        nc.sync.dma_start(
                            x_t[:],
                            xt[kc * 128 : (kc + 1) * 128, t * TN : (t + 1) * TN],
                        )
                        xts.append(x_t)
                    for kc in range(KC):
                        st, sp = kc == 0, kc == KC - 1
                        for m in range(2):  # Q then K, transposed layout
                            nc.tensor.matmul(
                                y_ps[:, m * 512 : (m + 1) * 512],
                                lhsT=w_sb[
                                    :,
                                    kc * WCOLS + m * 128 : kc * WCOLS + (m + 1) * 128,
                                ],
                                rhs=xts[kc][:],
                                start=st,
                                stop=sp,
                            )
                    # V natural layout: 4 token subtiles share one PSUM bank;
                    # start=True clears has_written flags bank-wide, so chain
                    # ordering deps so each accumulation group finishes before
                    # the next begins.
                    prev = None
                    for s in range(4):
                        for kc in range(KC):
                            st, sp = kc == 0, kc == KC - 1
                            mm = nc.tensor.matmul(
                                y_ps[:, 1024 + s * 128 : 1024 + (s + 1) * 128],
                                lhsT=xts[kc][:, s * 128 : (s + 1) * 128],
                                rhs=w_sb[:, kc * WCOLS + 256 : kc * WCOLS + WCOLS],
                                start=st,
                                stop=sp,
                            )
                            if prev is not None:
                                add_dep_helper(
                                    mm.ins, prev.ins, sync=False,
                                    reason="bank flag-clear order",
                                )
                            prev = mm
                    # epilogues on VectorE (keep ScalarE free for exp)
                    nc.vector.tensor_scalar_mul(
                        q_sb[:, t * TN : (t + 1) * TN], y_ps[:, 0:512], SCALE
                    )
                    nc.vector.tensor_copy(
                        k_sb[:, t * TN : (t + 1) * TN], y_ps[:, 512:1024]
                    )
                    nc.vector.tensor_copy(
                        v3[:, (t * 4) * HL : (t * 4 + 4) * HL, 0:64],
                        y_ps[:, 1024:1536]
                        .rearrange("p (s hd) -> p s hd", s=4)
                        .rearrange("p s (h d) -> p (s h) d", h=HL),
                    )
                    # early attention for qt=0 as K/V chunks become ready
                    if pv0 is None:
                        pv0 = [
                            psum.tile([128, 512], FP32, tag="pv", name=f"pv{h}")
                            for h in range(HL)
                        ]
                    avail = 4 * (i + 1)
                    while g_next < len(groups) and all(
                        kc < avail for kc, _ in groups[g_next]
                    ):
                        emit_group(groups[g_next], pv0, 0)
                        g_next += 1
                fin_q = [(pv0, 0)]

                # ---- attention for remaining q tiles ----
                for qt in range(1, NTOK // TN):
                    pv = [
                        psum.tile([128, 512], FP32, tag="pv", name=f"pv{h}")
                        for h in range(HL)
                    ]
                    for gi, g in enumerate(groups):
                        emit_group(g, pv, qt)
                        if gi == 0 and fin_q:
                            # the previous q-tile's last PV group was just
                            # flushed by the pipeline; it can be finished now
                            finish_qt(*fin_q.pop(0))
                    fin_q.append((pv, qt))
                while pending:
                    emit_pv_flush()
                for f in fin_q:
                    finish_qt(*f)
                fin_q = []

                # ---- output projection for batch b-1 (its A2A is long
                # done) -- emitted here so its PE matmuls never head-block
                # the in-order engine streams on a pending collective ----
                if b >= 1:
                    emit_outproj(b - 1)

                emit_a2a(b)

            emit_outproj(B - 1)

            if debug:
                for t in range(NT):
                    d1 = fin.tile([128, TN], FP32, tag="outt", name="d1")
                    nc.vector.tensor_copy(d1[:], q_sb[:, t * TN : (t + 1) * TN])
                    nc.sync.dma_start(qdump[:, t * TN : (t + 1) * TN], d1[:])
                    d2 = fin.tile([128, TN], FP32, tag="outt", name="d2")
                    nc.vector.tensor_copy(d2[:], k_sb[:, t * TN : (t + 1) * TN])
                    nc.sync.dma_start(kdump[:, t * TN : (t + 1) * TN], d2[:])
                nv = (T // 128) * HL * 65
                for j in range(0, nv, 1024):
                    wdt = min(1024, nv - j)
                    d3 = fin.tile([128, 1024], FP32, tag="outt", name="d3")
                    nc.vector.tensor_copy(d3[:, 0:wdt], v_sb[:, j : j + wdt])
                    nc.sync.dma_start(vdump[:, j : j + wdt], d3[:, 0:wdt])
                for i in range(N_CORES):
                    d4 = fin.tile([128, TPB], BF16, tag="outt", name="d4")
                    nc.sync.dma_start(d4[:], a2a_in[0][i, :, :])
                    d5 = fin.tile([128, TPB], FP32, tag="outt", name="d5")
                    nc.vector.tensor_copy(d5[:], d4[:])
                    nc.sync.dma_start(adump[i, :, :], d5[:])

    nc.compile()
    return nc


_NC_CACHE = None


def _get_nc():
    global _NC_CACHE
    if _NC_CACHE is None:
        _NC_CACHE = build_nc()
    return _NC_CACHE


def make_in_maps(x, w_qkv, w_out, b_out):
    x = np.asarray(x, dtype=np.float32)
    w_qkv = np.asarray(w_qkv, dtype=np.float32)
    w_out = np.asarray(w_out, dtype=np.float32)
    b_out = np.asarray(b_out, dtype=np.float32)

    xt_np = np.ascontiguousarray(x.reshape(T, D).T).astype(ml_dtypes.bfloat16)
    wo_np = np.ascontiguousarray(w_out.T).astype(ml_dtypes.bfloat16)
    b_np = np.ascontiguousarray(b_out.reshape(1, D))

    in_maps = []
    for c in range(N_CORES):
        rows = []
        for sec in range(3):  # q, k, v sections of w_qkv
            for hh in range(HL):
                h = HL * c + hh
                rows.append(w_qkv[sec * D + h * HD : sec * D + (h + 1) * HD, :])
        wt_np = np.ascontiguousarray(np.concatenate(rows, 0).T).astype(
            ml_dtypes.bfloat16
        )  # (1024, 384)
        in_maps.append({"xt": xt_np, "wt": wt_np, "wo": wo_np, "bias": b_np})
    return in_maps


def kernel(x, w_qkv, w_out, b_out, _trace=False, _tmpdir=None):
    in_maps = make_in_maps(x, w_qkv, w_out, b_out)
    nc = _get_nc()
    res = bass_utils.run_bass_kernel_spmd(
        nc, in_maps, core_ids=list(range(N_CORES)), trace=_trace, tmpdir=_tmpdir
    )
    # core j out row r = b*TPB + u -> global token b*NTOK + j*TPB + u
    stacked = np.stack([res.results[c]["out"] for c in range(N_CORES)], 0)
    full = stacked.reshape(N_CORES, B, TPB, D).transpose(1, 0, 2, 3)
    kernel.last_result = res
    return np.ascontiguousarray(full.reshape(B, NTOK, D)).astype(np.float32)


# revision 31
# speedup vs baseline: 1.0665x; 1.0045x over previous
"""Distributed multi-head attention kernel for 8 TRN2 NeuronCores.

Problem: x(4,2048,1024) -> qkv proj (w_qkv 3072x1024) -> 16-head attention
(head_dim 64, softmax) -> out proj (w_out 1024x1024 + b_out).

Sharding: head-parallel. Core c owns heads {2c, 2c+1}: it computes Q/K/V for
those heads over all 8192 tokens, runs attention, then a per-batch AllToAll
(1MB bf16) converts the head-sharded attention output into a token-sharded
layout (256 tokens/core/batch, all 16 heads) for the output projection --
no all-reduce needed. Work is issued per batch so phases pipeline: QKV(b+1)
and outproj(b-1) run under attention(b)'s softmax, which saturates ScalarE.

Per-core dataflow for batch b (all matmuls bf16 with f32 accumulate):
  1. QKV: Q^T/K^T = Wc @ X^T on PE ([128 = 2 heads x 64 dims, tokens] in
     SBUF, scale 1/8 folded into Q); V computed token-major (X chunks as the
     stationary operand) with a ones-column appended per 128-token chunk
     (65-wide blocks). The 4 V token-subtiles share one PSUM bank and
     start=True clears has_written flags bank-wide, so their accumulation
     groups are chained with explicit ordering deps. PSUM->SBUF epilogues
     run on VectorE to keep ScalarE free. Attention for q-tile 0 is
     interleaved with the QKV tiles as K/V chunks become available.
  2. Attention per (head, 512-wide q-tile): S^T tiles [128 k, 512 q] on PE
     (the two heads' matmuls auto-pack into PE row groups 0-63/64-127 and
     run concurrently); exp on ScalarE (PSUM->SBUF bf16, 1536-wide batches
     across 3 PSUM banks); P.V on PE with lhsT = V-chunk [128, 65]: the
     65th output row accumulates the softmax denominators for free. The PV
     matmuls are software-pipelined one exp-group behind the S matmuls
     (including across q-tile boundaries) so the in-order PE queue never
     head-stalls waiting for its own group's exp. A VectorE copy releases the PV PSUM
     bank; normalization runs off the critical path: the 512 denominators
     are DMA-reshaped across 128 partitions so reciprocal runs ~4
     elements/lane, then partition_broadcast (GpSimd) + multiply (VectorE).
  3. AllToAll over this batch's 8 x 256-token chunks.
  4. Out proj (pipelined one batch behind, so its PE matmuls never
     head-block the in-order engine queues on a pending collective):
     out = O^T.T @ w_out^T + b_out per 128-token tile, bias added on
     VectorE, DMA to the core's output slice.

Measured on 8 axon-tunneled trn2 cores: ~525 us HW exec, rel err 5.2e-3.
"""

import numpy as np
import ml_dtypes

import concourse.bass as bass
import concourse.mybir as mybir
import concourse.tile as tile
from concourse import bacc, bass_utils
from concourse.tile import add_dep_helper

FP32 = mybir.dt.float32
BF16 = mybir.dt.bfloat16
AF = mybir.ActivationFunctionType

N_CORES = 8
B, NTOK, D = 4, 2048, 1024
T = B * NTOK  # 8192 tokens total
NH, HD = 16, 64
HL = NH // N_CORES  # 2 heads per core
SCALE = float(HD) ** -0.5  # 0.125
TN = 512  # token tile for QKV / q tile for attention
NT = T // TN  # 16
KC = D // 128  # 8 contraction chunks for projections
KT = NTOK // 128  # 16 k-chunks per batch in attention
TPB = NTOK // N_CORES  # 256 tokens per (core, batch) after A2A
TPC = T // N_CORES  # 1024 tokens per core total
WCOLS = 3 * HL * HD  # 384 qkv output dims per core


def build_nc(debug=False):
    nc = bacc.Bacc(
        "TRN2", target_bir_lowering=False, debug=False, num_devices=N_CORES
    )
    xt = nc.dram_tensor("xt", [D, T], BF16, kind="ExternalInput").ap()
    wt = nc.dram_tensor("wt", [D, WCOLS], BF16, kind="ExternalInput").ap()
    wo = nc.dram_tensor("wo", [D, D], BF16, kind="ExternalInput").ap()
    bias = nc.dram_tensor("bias", [1, D], FP32, kind="ExternalInput").ap()
    # row r of out = batch r//TPB, token (core * TPB + r % TPB) of that batch
    out = nc.dram_tensor("out", [TPC, D], FP32, kind="ExternalOutput").ap()
    if debug:
        qdump = nc.dram_tensor("qdump", [128, T], FP32, kind="ExternalOutput").ap()
        kdump = nc.dram_tensor("kdump", [128, T], FP32, kind="ExternalOutput").ap()
        vdump = nc.dram_tensor(
            "vdump", [128, (T // 128) * HL * 65], FP32, kind="ExternalOutput"
        ).ap()
        adump = nc.dram_tensor(
            "adump", [N_CORES, HL * HD, TPB], FP32, kind="ExternalOutput"
        ).ap()

    with tile.TileContext(nc) as tc:
        with (
            tc.tile_pool(name="const", bufs=1) as const,
            tc.tile_pool(name="xin", bufs=24) as xin,
            tc.tile_pool(name="vtp", bufs=3) as vtp,
            tc.tile_pool(name="probs", bufs=6) as probs,
            tc.tile_pool(name="norm", bufs=6) as norm,
            tc.tile_pool(name="ot", bufs=6) as otp,
            tc.tile_pool(name="osb", bufs=2) as osbp,
            tc.tile_pool(name="fin", bufs=4) as fin,
            tc.tile_pool(name="psum", bufs=2, space="PSUM") as psum,
            tc.tile_pool(name="dram", bufs=1, space="DRAM") as dram,
        ):
            # ---- persistent SBUF state ----
            w_sb = const.tile([128, KC * WCOLS], BF16)
            nc.sync.dma_start(
                w_sb[:].rearrange("p (kc j) -> p kc j", kc=KC),
                wt.rearrange("(kc p) j -> p kc j", p=128),
            )
            wo_sb = const.tile([128, KC * D], BF16)
            nc.sync.dma_start(
                wo_sb[:].rearrange("p (kc n) -> p kc n", kc=KC),
                wo.rearrange("(kc p) n -> p kc n", p=128),
            )
            b_row = const.tile([1, D], FP32)
            nc.sync.dma_start(b_row[:], bias[:])
            bias_sb = const.tile([128, D], FP32)
            nc.gpsimd.partition_broadcast(bias_sb[:], b_row[:])

            q_sb = const.tile([128, T], BF16)  # [2 heads x 64, tokens] scaled
            k_sb = const.tile([128, T], BF16)
            # V token-major: [128 tok-in-chunk, (global chunk, head) x 65]
            v_sb = const.tile([128, (T // 128) * HL * 65], BF16)
            v3 = v_sb[:].rearrange("p (blk e) -> p blk e", e=65)
            nc.vector.memset(v3[:, :, 64:65], 1.0)

            a2a_in = {}
            a2a_out = {}
            for b in range(B - 1):
                a2a_in[b] = dram.tile(
                    [N_CORES, HL * HD, TPB], BF16, name=f"a2a_in{b}"
                )
                a2a_out[b] = dram.tile(
                    [N_CORES, HL * HD, TPB], BF16, name=f"a2a_out{b}"
                )
            # last batch: two half-size pieces so its collective and out-proj
            # overlap the tail of attention instead of serializing after it
            a2a_in3 = {}
            a2a_out3 = {}
            for hf in range(2):
                a2a_in3[hf] = dram.tile(
                    [N_CORES, HL * HD, 128], BF16, name=f"a2a_in3_{hf}"
                )
                a2a_out3[hf] = dram.tile(
                    [N_CORES, HL * HD, 128], BF16, name=f"a2a_out3_{hf}"
                )

            def emit_a2a(b):
                nc.gpsimd.collective_compute(
                    "AllToAll",
                    mybir.AluOpType.bypass,
                    replica_groups=[list(range(N_CORES))],
                    ins=[a2a_in[b].opt()],
                    outs=[a2a_out[b].opt()],
                )

            def emit_outproj(b):
                o_sb = osbp.tile([128, N_CORES * TPB], BF16, tag="osb", name="o_sb")
                for i in range(N_CORES):
                    nc.sync.dma_start(
                        o_sb[:, i * TPB : (i + 1) * TPB], a2a_out[b][i, :, :]
                    )
                for m in range(TPB // 128):
                    o_ps = [
                        psum.tile([128, 512], FP32, tag="pv", name=f"o_ps{nh}")
                        for nh in range(2)
                    ]
                    for i in range(N_CORES):
                        for nh in range(2):
                            nc.tensor.matmul(
                                o_ps[nh][:, :],
                                lhsT=o_sb[
                                    :, i * TPB + m * 128 : i * TPB + (m + 1) * 128
                                ],
                                rhs=wo_sb[:, i * D + nh * 512 : i * D + nh * 512 + 512],
                                start=(i == 0),
                                stop=(i == N_CORES - 1),
                            )
                    out_t = fin.tile([128, D], FP32, tag="outt", name="out_t")
                    for nh in range(2):
                        nc.vector.tensor_add(
                            out_t[:, nh * 512 : (nh + 1) * 512],
                            o_ps[nh][:, :],
                            bias_sb[:, nh * 512 : (nh + 1) * 512],
                        )
                    nc.sync.dma_start(
                        out[b * TPB + m * 128 : b * TPB + (m + 1) * 128, :],
                        out_t[:],
                    )

            for b in range(B):
                slot_list = [(kc, h) for kc in range(KT) for h in range(HL)]
                groups = [slot_list[g0 : g0 + 3] for g0 in range(0, len(slot_list), 3)]

                pending = []  # (group, p_t, pv) with S+exp emitted, PV not

                def emit_pv_flush(b=b):
                    group, p_t, pv = pending.pop(0)
                    for i, (kc, h) in enumerate(group):
                        gc = b * KT + kc
                        nc.tensor.matmul(
                            pv[h][0:65, :],
                            lhsT=v3[:, gc * HL + h, :],
                            rhs=p_t[:, i * 512 : (i + 1) * 512],
                            start=(kc == 0),
                            stop=(kc == KT - 1),
                        )

                def emit_group(group, pv, qt, b=b):
                    # S matmuls + exp for this group; the PV matmuls are
                    # emitted one group later (via pending) so the in-order
                    # PE queue never head-stalls on the exp of its own group
                    q_off = b * NTOK + qt * TN
                    width = len(group) * 512
                    s_t = psum.tile([128, 1536], FP32, tag="big3", name="s_t")
                    for i, (kc, h) in enumerate(group):
                        nc.tensor.matmul(
                            s_t[:, i * 512 : (i + 1) * 512],
                            lhsT=k_sb[
                                h * 64 : (h + 1) * 64,
                                b * NTOK + kc * 128 : b * NTOK + (kc + 1) * 128,
                            ],
                            rhs=q_sb[h * 64 : (h + 1) * 64, q_off : q_off + TN],
                            start=True,
                            stop=True,
                        )
                    p_t = probs.tile([128, 1536], BF16, tag="p", name="p_t")
                    nc.scalar.activation(p_t[:, 0:width], s_t[:, 0:width], AF.Exp)
                    pending.append((group, p_t, pv))
                    while len(pending) > 1:
                        emit_pv_flush()

                def finish_qt(pv, qt, b=b):
                    for h in range(HL):
                        # single copy releases the PV PSUM bank; the rest of
                        # the normalize chain runs on SBUF off the fast path
                        o_c = norm.tile([65, 512], FP32, tag="oc", name="o_c")
                        nc.vector.tensor_copy(o_c[:], pv[h][0:65, :])
                        # reciprocal on one partition is ~3.3us (512 sequential
                        # elements); DMA-reshape the 512 denominators across
                        # 128 partitions so it runs in ~4 elements/lane
                        rs = norm.tile([128, 4], FP32, tag="rs", name="rs")
                        nc.sync.dma_start(rs[:], o_c[64:65, :])
                        rr = norm.tile([128, 4], FP32, tag="rr", name="rr")
                        nc.vector.reciprocal(rr[:], rs[:])
                        rec = norm.tile([1, 512], FP32, tag="rec", name="rec")
                        nc.sync.dma_start(rec[:], rr[:])
                        bc = norm.tile([64, 512], FP32, tag="bc", name="bc")
                        nc.gpsimd.partition_broadcast(bc[:], rec[:])
                        o_t = otp.tile([64, 512], BF16, tag="o", name="o_t")
                        nc.vector.tensor_mul(o_t[:], o_c[0:64, :], bc[:])
                        if b < B - 1:
                            nc.sync.dma_start(
                                a2a_in[b][
                                    2 * qt : 2 * qt + 2, h * 64 : (h + 1) * 64, :
                                ].rearrange("j p e -> p j e"),
                                o_t[:].rearrange("p (j e) -> p j e", j=2),
                            )
                        else:
                            j0 = (qt % 2) * 4
                            nc.sync.dma_start(
                                a2a_in3[qt // 2][
                                    j0 : j0 + 4, h * 64 : (h + 1) * 64, :
                                ].rearrange("j p e -> p j e"),
                                o_t[:].rearrange("p (j e) -> p j e", j=4),
                            )

                # ---- QKV for this batch, interleaved with qt=0 attention ----
                pv0 = None
                g_next = 0
                for i, t in enumerate(range(4 * b, 4 * b + 4)):
                    y_ps = psum.tile([128, 1536], FP32, tag="big3", name="y_ps")
                    xts = []
                    for kc in range(KC):
                        x_t = xin.tile([128, TN], BF16, tag="xt", name="x_t")
                        nc.sync.dma_start(
                            x_t[:],
                            xt[kc * 128 : (kc + 1) * 128, t * TN : (t + 1) * TN],
                        )
                        xts.append(x_t)
                    for kc in range(KC):
                        st, sp = kc == 0, kc == KC - 1
                        for m in range(2):  # Q then K, transposed layout
                            nc.tensor.matmul(
                                y_ps[:, m * 512 : (m + 1) * 512],
                                lhsT=w_sb[
                                    :,
                                    kc * WCOLS + m * 128 : kc * WCOLS + (m + 1) * 128,
                                ],
                                rhs=xts[kc][:],
                                start=st,
                                stop=sp,
                            )
                    # V natural layout: 4 token subtiles share one PSUM bank;
                    # start=True clears has_written flags bank-wide, so chain
                    # ordering deps so each accumulation group finishes before
                    # the next begins.
                    prev = None
                    for s in range(4):
                        for kc in range(KC):
                            st, sp = kc == 0, kc == KC - 1
                            mm = nc.tensor.matmul(
                                y_ps[:, 1024 + s * 128 : 1024 + (s + 1) * 128],
                                lhsT=xts[kc][:, s * 128 : (s + 1) * 128],
                                rhs=w_sb[:, kc * WCOLS + 256 : kc * WCOLS + WCOLS],
                                start=st,
                                stop=sp,
                            )
                            if prev is not None:
                                add_dep_helper(
                                    mm.ins, prev.ins, sync=False,
                                    reason="bank flag-clear order",
                                )
                            prev = mm
                    # epilogues on VectorE (keep ScalarE free for exp)
                    nc.vector.tensor_scalar_mul(
                        q_sb[:, t * TN : (t + 1) * TN], y_ps[:, 0:512], SCALE
                    )
                    nc.vector.tensor_copy(
                        k_sb[:, t * TN : (t + 1) * TN], y_ps[:, 512:1024]
                    )
                    nc.vector.tensor_copy(
                        v3[:, (t * 4) * HL : (t * 4 + 4) * HL, 0:64],
                        y_ps[:, 1024:1536]
                        .rearrange("p (s hd) -> p s hd", s=4)
                        .rearrange("p s (h d) -> p (s h) d", h=HL),
                    )
                    # early attention for qt=0 as K/V chunks become ready
                    if pv0 is None:
                        pv0 = [
                            psum.tile([128, 512], FP32, tag="pv", name=f"pv{h}")
                            for h in range(HL)
                        ]
                    avail = 4 * (i + 1)
                    while g_next < len(groups) and all(
                        kc < avail for kc, _ in groups[g_next]
                    ):
                        emit_group(groups[g_next], pv0, 0)
                        g_next += 1
                fin_q = [(pv0, 0)]

                # ---- attention for remaining q tiles ----
                for qt in range(1, NTOK // TN):
                    pv = [
                        psum.tile([128, 512], FP32, tag="pv", name=f"pv{h}")
                        for h in range(HL)
                    ]
                    for gi, g in enumerate(groups):
                        emit_group(g, pv, qt)
                        if gi == 0 and fin_q:
                            # the previous q-tile's last PV group was just
                            # flushed by the pipeline; it can be finished now
                            finish_qt(*fin_q.pop(0))
                            if b == B - 1 and qt == 2:
                                # qt0+qt1 of the last batch are complete
                                nc.gpsimd.collective_compute(
                                    "AllToAll",
                                    mybir.AluOpType.bypass,
                                    replica_groups=[list(range(N_CORES))],
                                    ins=[a2a_in3[0].opt()],
                                    outs=[a2a_out3[0].opt()],
                                )
                    fin_q.append((pv, qt))
                while pending:
                    emit_pv_flush()
                for f in fin_q:
                    finish_qt(*f)
                fin_q = []

                # ---- output projection for batch b-1 (its A2A is long
                # done) -- emitted here so its PE matmuls never head-block
                # the in-order engine streams on a pending collective ----
                if b >= 1:
                    emit_outproj(b - 1)

                if b < B - 1:
                    emit_a2a(b)

            nc.gpsimd.collective_compute(
                "AllToAll",
                mybir.AluOpType.bypass,
                replica_groups=[list(range(N_CORES))],
                ins=[a2a_in3[1].opt()],
                outs=[a2a_out3[1].opt()],
            )
            for m in range(2):
                o_sbh = osbp.tile(
                    [128, N_CORES * 128], BF16, tag="osb", name="o_sbh"
                )
                for i in range(N_CORES):
                    nc.sync.dma_start(
                        o_sbh[:, i * 128 : (i + 1) * 128], a2a_out3[m][i, :, :]
                    )
                o_ps = [
                    psum.tile([128, 512], FP32, tag="pv", name=f"o_ps{nh}")
                    for nh in range(2)
                ]
                for i in range(N_CORES):
                    for nh in range(2):
                        nc.tensor.matmul(
                            o_ps[nh][:, :],
                            lhsT=o_sbh[:, i * 128 : (i + 1) * 128],
                            rhs=wo_sb[:, i * D + nh * 512 : i * D + nh * 512 + 512],
                            start=(i == 0),
                            stop=(i == N_CORES - 1),
                        )
                out_t = fin.tile([128, D], FP32, tag="outt", name="out_t")
                for nh in range(2):
                    nc.vector.tensor_add(
                        out_t[:, nh * 512 : (nh + 1) * 512],
                        o_ps[nh][:, :],
                        bias_sb[:, nh * 512 : (nh + 1) * 512],
                    )
                nc.sync.dma_start(
                    out[(B - 1) * TPB + m * 128 : (B - 1) * TPB + (m + 1) * 128, :],
                    out_t[:],
                )

            if debug:
                for t in range(NT):
                    d1 = fin.tile([128, TN], FP32, tag="outt", name="d1")
                    nc.vector.tensor_copy(d1[:], q_sb[:, t * TN : (t + 1) * TN])
                    nc.sync.dma_start(qdump[:, t * TN : (t + 1) * TN], d1[:])
                    d2 = fin.tile([128, TN], FP32, tag="outt", name="d2")
                    nc.vector.tensor_copy(d2[:], k_sb[:, t * TN : (t + 1) * TN])
                    nc.sync.dma_start(kdump[:, t * TN : (t + 1) * TN], d2[:])
                nv = (T // 128) * HL * 65
                for j in range(0, nv, 1024):
                    wdt = min(1024, nv - j)
                    d3 = fin.tile([128, 1024], FP32, tag="outt", name="d3")
                    nc.vector.tensor_copy(d3[:, 0:wdt], v_sb[:, j : j + wdt])
                    nc.sync.dma_start(vdump[:, j : j + wdt], d3[:, 0:wdt])
                for i in range(N_CORES):
                    d4 = fin.tile([128, TPB], BF16, tag="outt", name="d4")
                    nc.sync.dma_start(d4[:], a2a_in[0][i, :, :])
                    d5 = fin.tile([128, TPB], FP32, tag="outt", name="d5")
                    nc.vector.tensor_copy(d5[:], d4[:])
                    nc.sync.dma_start(adump[i, :, :], d5[:])

    nc.compile()
    return nc


_NC_CACHE = None


def _get_nc():
    global _NC_CACHE
    if _NC_CACHE is None:
        _NC_CACHE = build_nc()
    return _NC_CACHE


def make_in_maps(x, w_qkv, w_out, b_out):
    x = np.asarray(x, dtype=np.float32)
    w_qkv = np.asarray(w_qkv, dtype=np.float32)
    w_out = np.asarray(w_out, dtype=np.float32)
    b_out = np.asarray(b_out, dtype=np.float32)

    xt_np = np.ascontiguousarray(x.reshape(T, D).T).astype(ml_dtypes.bfloat16)
    wo_np = np.ascontiguousarray(w_out.T).astype(ml_dtypes.bfloat16)
    b_np = np.ascontiguousarray(b_out.reshape(1, D))

    in_maps = []
    for c in range(N_CORES):
        rows = []
        for sec in range(3):  # q, k, v sections of w_qkv
            for hh in range(HL):
                h = HL * c + hh
                rows.append(w_qkv[sec * D + h * HD : sec * D + (h + 1) * HD, :])
        wt_np = np.ascontiguousarray(np.concatenate(rows, 0).T).astype(
            ml_dtypes.bfloat16
        )  # (1024, 384)
        in_maps.append({"xt": xt_np, "wt": wt_np, "wo": wo_np, "bias": b_np})
    return in_maps


def kernel(x, w_qkv, w_out, b_out, _trace=False, _tmpdir=None):
    in_maps = make_in_maps(x, w_qkv, w_out, b_out)
    nc = _get_nc()
    res = bass_utils.run_bass_kernel_spmd(
        nc, in_maps, core_ids=list(range(N_CORES)), trace=_trace, tmpdir=_tmpdir
    )
    # core j out rows: batches 0-2: r = b*256+u -> token b*2048 + j*256 + u;
    # batch 3 (half-split A2A): r = 768 + hf*128 + u -> 6144 + hf*1024 + j*128 + u
    full = np.empty((T, D), np.float32)
    for j in range(N_CORES):
        o = np.asarray(res.results[j]["out"], dtype=np.float32)
        for b in range(B - 1):
            full[b * NTOK + j * TPB : b * NTOK + (j + 1) * TPB] = o[
                b * TPB : (b + 1) * TPB
            ]
        for hf in range(2):
            dst = (B - 1) * NTOK + hf * 1024 + j * 128
            srcr = (B - 1) * TPB + hf * 128
            full[dst : dst + 128] = o[srcr : srcr + 128]
    kernel.last_result = res
    return full.reshape(B, NTOK, D)
